# revision 5
# baseline (speedup 1.0000x reference)
"""Trainium2 Bass kernel for an LSTM cell (DPLSTMCell).

  gates = input @ W_ih^T + b_ih + h_0 @ W_hh^T + b_hh          [B, 4H]
  i, f, g, o = split(gates, 4)
  c_1 = sigmoid(f) * c_0 + sigmoid(i) * tanh(g)
  h_1 = sigmoid(o) * tanh(c_1)

B=16384, IN=H=1024. Data-parallel across 8 NeuronCores: each core gets a
2048-row batch shard; weights/biases are replicated.

Mixed-precision matmuls, selected per gate by LSTM_SCHEME (default "AAEA",
gate order i,f,g,o):
  'A' — fp8 e4m3 with DoubleRow perf mode (2 k-tiles per pass). Weights are
        pre-scaled by 32 on the host so U(-1/32,1/32) entries clear e4m3's
        denormal floor (2^-9); the 1/32 is folded into the ACT epilogue's
        scale operand, and the gate's bias row is pre-scaled by 32.
  'E' — bf16, one k-tile per matmul.
The default keeps the tanh (cell) gate in bf16: its slope-1 nonlinearity
dominates the quantization error budget (sim: AAEA 1.55e-2 vs AAAA 2.42e-2,
threshold 2e-2).

Device layout per core (only tensors needed by the scheme are declared):
  xT8/hT8 [128, MT, KT, 128] f8e4 : xT8[p, m, kt, b] = x[m*128 + b, kt*128 + p]
  xTb/hTb [128, MT, KT, 128] bf16 : same layout
  w8ih/w8hh [128, KT, 2, nA, 512] f8e4 : 32*W[qa*1024 + j*512 + s, kt*128 + p]
  wbih/wbhh [128, KT, 2, nE, 512] bf16 : W[qe*1024 + j*512 + s, kt*128 + p]
  bias [1, 2, 4, 512] f32 : (b_ih + b_hh) with 'A' gates scaled by 32
  c0 [2048, 1024] bf16; h1/c1 [2048, 1024] bf16 (upcast to f32 on host).

Per batch-tile m (128 rows) and gate-column group j (512 of 1024 columns):
4 PSUM banks (i, f, g, o). 'A' banks accumulate 4+4 DoubleRow passes of
[128k x 2 x 128b]^T (x) [128k x 2 x 512g]; 'E' banks 8+8 bf16 matmuls. The
fp32 bias (DMA-broadcast across partitions once) is added on DVE during the
PSUM->SBUF move, ACT applies sigmoid/tanh (scale=1/32 on 'A' gates), and DVE
forms c_1 / h_1 in bf16.
"""

import os
import sys

import numpy as np

for _p in ("/opt/trn_rl_repo", "/root/.axon_site/_ro/trn_rl_repo"):
    if os.path.isdir(_p) and _p not in sys.path:
        sys.path.append(_p)

import ml_dtypes  # noqa: E402

import concourse.bass as bass  # noqa: E402
import concourse.mybir as mybir  # noqa: E402
import concourse.tile as tile  # noqa: E402
from concourse.bass_utils import run_bass_kernel_spmd  # noqa: E402

N_CORES = 8
B = 16384
IN = 1024
H = 1024
BL = B // N_CORES  # 2048 rows per core
MT = BL // 128     # 16 batch tiles per core
KT = IN // 128     # 8 k-tiles
NQ = 512           # free dim per PSUM bank
BF16 = ml_dtypes.bfloat16
F8E4 = ml_dtypes.float8_e4m3
WSCALE = 32.0      # host-side premultiplier on fp8 weights

SCHEME = os.environ.get("LSTM_SCHEME", "AAEA")  # per-gate i,f,g,o: A=fp8, E=bf16

# The walrus in this container only accepts one sync-wait command per
# instruction; Tile emits instructions (notably the final drain) with more.
_MAX_WAITS_PER_INST = 1


def _split_excess_waits(nc, cap=_MAX_WAITS_PER_INST):
    """Move excess sem-waits onto NoOps inserted ahead of the instruction
    (same engine). Waits are AND-conditions on monotonically increasing
    semaphores, so satisfying them one-by-one is equivalent."""
    for f in nc.m.functions:
        for blk in f.blocks:
            new_insts = []
            for inst in blk.instructions:
                si = getattr(inst, "sync_info", None)
                if si is not None and si.on_wait and len(si.on_wait) > cap:
                    waits = list(si.on_wait)
                    extra, keep = waits[:-cap], waits[-cap:]
                    while extra:
                        chunk, extra = extra[:cap], extra[cap:]
                        new_insts.append(
                            mybir.InstNoOp(
                                name=nc.get_next_instruction_name(),
                                sync_info=mybir.SyncInfo(on_wait=chunk, on_update=[]),
                                bass_nofuse=True,
                                engine=inst.engine,
                            )
                        )
                    inst.sync_info = mybir.SyncInfo(
                        on_wait=keep, on_update=list(si.on_update or [])
                    )
                new_insts.append(inst)
            blk.instructions[:] = new_insts


def _dedupe_ldweights(nc):
    """Remove an InstLdweights whose weights AP matches the previous
    InstLdweights on PE, with only InstMatmult in between — the PE array
    still holds those weights, so the reload is redundant. Only drops
    instructions with no semaphore waits/updates."""
    n = 0
    for f in nc.m.functions:
        for blk in f.blocks:
            prev_key = None
            keep = []
            for inst in blk.instructions:
                if getattr(inst, "engine", None) != mybir.EngineType.PE:
                    keep.append(inst)
                    continue
                tn = type(inst).__name__
                if tn == "InstLdweights":
                    w = inst.ins[0]
                    key = (w.memref, w.offset, str(w.ap), str(w.dtype),
                           str(getattr(inst, "perf_mode", None)))
                    si = getattr(inst, "sync_info", None)
                    clean = si is None or (not si.on_wait and not si.on_update)
                    if key == prev_key and clean:
                        n += 1
                        continue  # drop it
                    prev_key = key
                elif tn != "InstMatmult":
                    prev_key = None
                keep.append(inst)
            blk.instructions[:] = keep
    return n


def _build_nc(repeat=None):
    """repeat>1 wraps the whole body in a hardware loop — benchmarking only
    (outputs are simply rewritten each iteration)."""
    if repeat is None:
        repeat = int(os.environ.get("LSTM_BENCH_REPEAT", "1"))
    scheme = list(SCHEME)
    assert len(scheme) == 4 and all(s in "AE" for s in scheme), scheme
    qa_idx = {q: i for i, q in enumerate([q for q in range(4) if scheme[q] == "A"])}
    qe_idx = {q: i for i, q in enumerate([q for q in range(4) if scheme[q] == "E"])}
    nA, nE = len(qa_idx), len(qe_idx)

    nc = bass.Bass()
    f32 = mybir.dt.float32
    bf16 = mybir.dt.bfloat16
    f8e4 = mybir.dt.float8e4
    SIG = mybir.ActivationFunctionType.Sigmoid
    TANH = mybir.ActivationFunctionType.Tanh
    DR = mybir.MatmulPerfMode.DoubleRow
    GATE_FUNC = [SIG, SIG, TANH, SIG]

    if nA:
        xT8 = nc.declare_dram_parameter("xT8", [128, MT, KT, 128], f8e4, isOutput=False)
        hT8 = nc.declare_dram_parameter("hT8", [128, MT, KT, 128], f8e4, isOutput=False)
        w8ih = nc.declare_dram_parameter(
            "w8ih", [128, KT, 2, nA, NQ], f8e4, isOutput=False
        )
        w8hh = nc.declare_dram_parameter(
            "w8hh", [128, KT, 2, nA, NQ], f8e4, isOutput=False
        )
    if nE:
        xTb = nc.declare_dram_parameter("xTb", [128, MT, KT, 128], bf16, isOutput=False)
        hTb = nc.declare_dram_parameter("hTb", [128, MT, KT, 128], bf16, isOutput=False)
        wbih = nc.declare_dram_parameter(
            "wbih", [128, KT, 2, nE, NQ], bf16, isOutput=False
        )
        wbhh = nc.declare_dram_parameter(
            "wbhh", [128, KT, 2, nE, NQ], bf16, isOutput=False
        )
    c0 = nc.declare_dram_parameter("c0", [BL, H], bf16, isOutput=False)
    bjqs = nc.declare_dram_parameter("bjqs", [1, 2, 4, NQ], f32, isOutput=False)
    h1 = nc.declare_dram_parameter("h1", [BL, H], bf16, isOutput=True)
    c1 = nc.declare_dram_parameter("c1", [BL, H], bf16, isOutput=True)

    with tile.TileContext(nc) as tc:
        with (
            tc.tile_pool(name="w", bufs=1) as wpool,
            tc.tile_pool(name="xh", bufs=4) as xhpool,
            tc.tile_pool(name="cc", bufs=4) as cpool,
            tc.tile_pool(name="act", bufs=2) as apool,
            tc.tile_pool(name="outp", bufs=4) as opool,
            tc.tile_pool(name="ps", bufs=8, space="PSUM") as pspool,
        ):
            if nA:
                w8ih_sb = wpool.tile([128, KT, 2, nA, NQ], f8e4)
                w8hh_sb = wpool.tile([128, KT, 2, nA, NQ], f8e4)
            if nE:
                wbih_sb = wpool.tile([128, KT, 2, nE, NQ], bf16)
                wbhh_sb = wpool.tile([128, KT, 2, nE, NQ], bf16)
            bias_sb = wpool.tile([128, 2, 4, NQ], f32)

            if repeat > 1:
                loop_cm = tc.For_i(0, repeat, 1)
                loop_cm.__enter__()

            # Weights on the SP HWDGE queue in exact consumption order
            # (per j: fp8 gates, then bf16 gates; x-matrix then h-matrix).
            # The first fp8 chunk is split so the opening matmul's k-tile
            # pair lands in SBUF as early as possible.
            for j in range(2):
                for qs in range(nA):
                    if j == 0 and qs == 0:
                        nc.sync.dma_start(
                            out=w8ih_sb[:, 0:2, j, qs], in_=w8ih[:, 0:2, j, qs]
                        )
                        nc.sync.dma_start(
                            out=w8ih_sb[:, 2:, j, qs], in_=w8ih[:, 2:, j, qs]
                        )
                    else:
                        nc.sync.dma_start(
                            out=w8ih_sb[:, :, j, qs], in_=w8ih[:, :, j, qs]
                        )
                    nc.sync.dma_start(
                        out=w8hh_sb[:, :, j, qs], in_=w8hh[:, :, j, qs]
                    )
                for qs in range(nE):
                    for kh in range(2):
                        ks = slice(kh * 4, (kh + 1) * 4)
                        nc.sync.dma_start(
                            out=wbih_sb[:, ks, j, qs], in_=wbih[:, ks, j, qs]
                        )
                        nc.sync.dma_start(
                            out=wbhh_sb[:, ks, j, qs], in_=wbhh[:, ks, j, qs]
                        )

            for m in range(MT):
                if nA:
                    xm8 = xhpool.tile([128, KT, 128], f8e4, tag="xm8")
                    hm8 = xhpool.tile([128, KT, 128], f8e4, tag="hm8")
                    nc.scalar.dma_start(out=xm8, in_=xT8[:, m])
                    nc.scalar.dma_start(out=hm8, in_=hT8[:, m])
                if nE:
                    xmb = xhpool.tile([128, KT, 128], bf16, tag="xmb")
                    hmb = xhpool.tile([128, KT, 128], bf16, tag="hmb")
                    nc.scalar.dma_start(out=xmb, in_=xTb[:, m])
                    nc.scalar.dma_start(out=hmb, in_=hTb[:, m])
                if m == 0:
                    # bias isn't needed until the first matmul group finishes;
                    # keep it behind the first x/h tiles on the ACT queue.
                    bj_ap = bjqs[:]
                    bias_bcast = bass.AP(
                        tensor=bj_ap.tensor,
                        offset=bj_ap.offset,
                        ap=[[0, 128]] + list(bj_ap.ap[1:]),
                    )
                    nc.scalar.dma_start(out=bias_sb, in_=bias_bcast)
                for j in range(2):
                    cs = slice(j * NQ, (j + 1) * NQ)
                    rs = slice(m * 128, (m + 1) * 128)

                    c0t = cpool.tile([128, NQ], bf16, tag="c0")
                    nc.scalar.dma_start(out=c0t, in_=c0[rs, cs])

                    ps = [
                        pspool.tile([128, NQ], f32, tag="ps", name=f"ps{q}")
                        for q in range(4)
                    ]
                    # fp8 DoubleRow chains: stationary x/h k-tile pair shared
                    # across the fp8 banks (amortized ldweights when deduped).
                    for kp in range(KT // 2):
                        kslc = slice(2 * kp, 2 * kp + 2)
                        for q in range(4):
                            if scheme[q] != "A":
                                continue
                            nc.tensor.matmul(
                                ps[q],
                                lhsT=xm8[:, kslc],
                                rhs=w8ih_sb[:, kslc, j, qa_idx[q]],
                                start=(kp == 0), stop=False,
                                perf_mode=DR, skip_group_check=True,
                            )
                    for kp in range(KT // 2):
                        kslc = slice(2 * kp, 2 * kp + 2)
                        last = kp == KT // 2 - 1
                        for q in range(4):
                            if scheme[q] != "A":
                                continue
                            nc.tensor.matmul(
                                ps[q],
                                lhsT=hm8[:, kslc],
                                rhs=w8hh_sb[:, kslc, j, qa_idx[q]],
                                start=False, stop=last,
                                perf_mode=DR, skip_group_check=True,
                            )
                    # bf16 chains
                    for kt in range(KT):
                        for q in range(4):
                            if scheme[q] != "E":
                                continue
                            nc.tensor.matmul(
                                ps[q],
                                lhsT=xmb[:, kt],
                                rhs=wbih_sb[:, kt, j, qe_idx[q]],
                                start=(kt == 0), stop=False,
                                skip_group_check=True,
                            )
                    for kt in range(KT):
                        last = kt == KT - 1
                        for q in range(4):
                            if scheme[q] != "E":
                                continue
                            nc.tensor.matmul(
                                ps[q],
                                lhsT=hmb[:, kt],
                                rhs=wbhh_sb[:, kt, j, qe_idx[q]],
                                start=False, stop=last,
                                skip_group_check=True,
                            )

                    g = [
                        apool.tile([128, NQ], bf16, tag=f"g{q}", name=f"g{q}")
                        for q in range(4)
                    ]
                    for q in range(4):
                        # bias add on DVE (PSUM -> SBUF); ACT applies the
                        # nonlinearity, folding the fp8 weight prescale back
                        # out via its input scale operand. bf16 gate tiles
                        # halve DVE/ACT SBUF traffic; the rounding is far
                        # below the fp8 matmul quantization noise.
                        nc.vector.tensor_add(
                            out=g[q], in0=ps[q], in1=bias_sb[:, j, q]
                        )
                        nc.scalar.activation(
                            out=g[q], in_=g[q], func=GATE_FUNC[q],
                            scale=(1.0 / WSCALE) if scheme[q] == "A" else 1.0,
                        )

                    gi, gf, gg, go = g
                    nc.vector.tensor_mul(out=gi, in0=gi, in1=gg)   # sig(i)*tanh(g)
                    nc.vector.tensor_mul(out=gf, in0=gf, in1=c0t)  # sig(f)*c0
                    c1t = opool.tile([128, NQ], bf16, tag="c1")
                    nc.vector.tensor_add(out=c1t, in0=gi, in1=gf)
                    tc1 = apool.tile([128, NQ], bf16, tag="tc1")
                    nc.scalar.activation(out=tc1, in_=c1t, func=TANH)
                    h1t = opool.tile([128, NQ], bf16, tag="h1")
                    nc.vector.tensor_mul(out=h1t, in0=go, in1=tc1)

                    nc.sync.dma_start(out=c1[rs, cs], in_=c1t)
                    nc.sync.dma_start(out=h1[rs, cs], in_=h1t)

            if repeat > 1:
                loop_cm.__exit__(None, None, None)

    _split_excess_waits(nc)
    if os.environ.get("LSTM_LDW_DEDUPE", "0") == "1":
        _dedupe_ldweights(nc)
    return nc


_NC = None


def _get_nc():
    global _NC
    if _NC is None:
        _NC = _build_nc()
    return _NC


def _prep_xT4(x, dtype):
    """[B, 1024] fp32 -> [N_CORES][128, MT, KT, 128] per-core arrays."""
    v = x.reshape(N_CORES, MT, 128, KT, 128)  # [c, m, b, kt, p]
    v = v.transpose(0, 4, 1, 3, 2)            # [c, p, m, kt, b]
    v = v.astype(dtype)
    return [np.ascontiguousarray(v[c]) for c in range(N_CORES)]


def _prep_w5(w, gates, dtype, scale=1.0):
    """[4096, 1024] fp32 -> [128, KT, 2, len(gates), 512] (replicated)."""
    v = w.reshape(4, 2, NQ, KT, 128)[gates]  # [q, j, s, kt, p]
    if scale != 1.0:
        v = v * scale
    v = v.transpose(4, 3, 1, 0, 2)           # [p, kt, j, q, s]
    return np.ascontiguousarray(v.astype(dtype))


def _make_in_maps(input, h_0, c_0, W_ih, b_ih, W_hh, b_hh):
    scheme = list(SCHEME)
    gates_a = [q for q in range(4) if scheme[q] == "A"]
    gates_e = [q for q in range(4) if scheme[q] == "E"]

    x = np.asarray(input, dtype=np.float32)
    h0 = np.asarray(h_0, dtype=np.float32)
    c0 = np.asarray(c_0, dtype=np.float32)
    wih = np.asarray(W_ih, dtype=np.float32)
    whh = np.asarray(W_hh, dtype=np.float32)
    b = (np.asarray(b_ih, dtype=np.float32) + np.asarray(b_hh, dtype=np.float32))
    bq = b.reshape(4, 2, NQ).copy()
    for q in gates_a:
        bq[q] *= WSCALE
    bjqs = np.ascontiguousarray(bq.transpose(1, 0, 2)[None])  # [1, 2(j), 4(q), 512]

    base = {
        "bjqs": bjqs,
    }
    if gates_a:
        base["w8ih"] = _prep_w5(wih, gates_a, F8E4, WSCALE)
        base["w8hh"] = _prep_w5(whh, gates_a, F8E4, WSCALE)
        xs8 = _prep_xT4(x, F8E4)
        hs8 = _prep_xT4(h0, F8E4)
    if gates_e:
        base["wbih"] = _prep_w5(wih, gates_e, BF16)
        base["wbhh"] = _prep_w5(whh, gates_e, BF16)
        xsb = _prep_xT4(x, BF16)
        hsb = _prep_xT4(h0, BF16)
    c0s = c0.reshape(N_CORES, BL, H).astype(BF16)

    in_maps = []
    for c in range(N_CORES):
        m = dict(base)
        m["c0"] = np.ascontiguousarray(c0s[c])
        if gates_a:
            m["xT8"] = xs8[c]
            m["hT8"] = hs8[c]
        if gates_e:
            m["xTb"] = xsb[c]
            m["hTb"] = hsb[c]
        in_maps.append(m)
    return in_maps


def kernel(input, h_0, c_0, W_ih, b_ih, W_hh, b_hh):
    in_maps = _make_in_maps(input, h_0, c_0, W_ih, b_ih, W_hh, b_hh)
    nc = _get_nc()
    res = run_bass_kernel_spmd(nc, in_maps, core_ids=list(range(N_CORES)))
    h_1 = np.concatenate(
        [res.results[c]["h1"].astype(np.float32) for c in range(N_CORES)], axis=0
    )
    c_1 = np.concatenate(
        [res.results[c]["c1"].astype(np.float32) for c in range(N_CORES)], axis=0
    )
    return (h_1, c_1)


# revision 6
# speedup vs baseline: 1.0467x; 1.0467x over previous
"""Trainium2 Bass kernel for an LSTM cell (DPLSTMCell).

  gates = input @ W_ih^T + b_ih + h_0 @ W_hh^T + b_hh          [B, 4H]
  i, f, g, o = split(gates, 4)
  c_1 = sigmoid(f) * c_0 + sigmoid(i) * tanh(g)
  h_1 = sigmoid(o) * tanh(c_1)

B=16384, IN=H=1024. Data-parallel across 8 NeuronCores: each core gets a
2048-row batch shard; weights/biases are replicated.

Mixed-precision matmuls, selected per gate by LSTM_SCHEME (default "AAEA",
gate order i,f,g,o):
  'A' — fp8 e4m3 with DoubleRow perf mode (2 k-tiles per pass). Weights are
        pre-scaled by 32 on the host so U(-1/32,1/32) entries clear e4m3's
        denormal floor (2^-9); the 1/32 is folded into the ACT epilogue's
        scale operand, and the gate's bias row is pre-scaled by 32.
  'E' — bf16, one k-tile per matmul.
The default keeps the tanh (cell) gate in bf16: its slope-1 nonlinearity
dominates the quantization error budget (sim: AAEA 1.55e-2 vs AAAA 2.42e-2,
threshold 2e-2).

Device layout per core (only tensors needed by the scheme are declared):
  xT8/hT8 [128, MT, KT, 128] f8e4 : xT8[p, m, kt, b] = x[m*128 + b, kt*128 + p]
  xTb/hTb [128, MT, KT, 128] bf16 : same layout
  w8ih/w8hh [128, KT, 2, nA, 512] f8e4 : 32*W[qa*1024 + j*512 + s, kt*128 + p]
  wbih/wbhh [128, KT, 2, nE, 512] bf16 : W[qe*1024 + j*512 + s, kt*128 + p]
  bias [1, 2, 4, 512] f32 : (b_ih + b_hh) with 'A' gates scaled by 32
  c0 [2048, 1024] bf16; h1/c1 [2048, 1024] bf16 (upcast to f32 on host).

Per batch-tile m (128 rows) and gate-column group j (512 of 1024 columns):
4 PSUM banks (i, f, g, o). 'A' banks accumulate 4+4 DoubleRow passes of
[128k x 2 x 128b]^T (x) [128k x 2 x 512g]; 'E' banks 8+8 bf16 matmuls. The
fp32 bias (DMA-broadcast across partitions once) is added on DVE during the
PSUM->SBUF move, ACT applies sigmoid/tanh (scale=1/32 on 'A' gates), and DVE
forms c_1 / h_1 in bf16.
"""

import os
import sys

import numpy as np

for _p in ("/opt/trn_rl_repo", "/root/.axon_site/_ro/trn_rl_repo"):
    if os.path.isdir(_p) and _p not in sys.path:
        sys.path.append(_p)

import ml_dtypes  # noqa: E402

import concourse.bass as bass  # noqa: E402
import concourse.mybir as mybir  # noqa: E402
import concourse.tile as tile  # noqa: E402
from concourse.bass_utils import run_bass_kernel_spmd  # noqa: E402

N_CORES = 8
B = 16384
IN = 1024
H = 1024
BL = B // N_CORES  # 2048 rows per core
MT = BL // 128     # 16 batch tiles per core
KT = IN // 128     # 8 k-tiles
NQ = 512           # free dim per PSUM bank
BF16 = ml_dtypes.bfloat16
F8E4 = ml_dtypes.float8_e4m3
WSCALE = 32.0      # host-side premultiplier on fp8 weights

SCHEME = os.environ.get("LSTM_SCHEME", "AAEA")  # per-gate i,f,g,o: A=fp8, E=bf16

# The walrus in this container only accepts one sync-wait command per
# instruction; Tile emits instructions (notably the final drain) with more.
_MAX_WAITS_PER_INST = 1


def _split_excess_waits(nc, cap=_MAX_WAITS_PER_INST):
    """Move excess sem-waits onto NoOps inserted ahead of the instruction
    (same engine). Waits are AND-conditions on monotonically increasing
    semaphores, so satisfying them one-by-one is equivalent."""
    for f in nc.m.functions:
        for blk in f.blocks:
            new_insts = []
            for inst in blk.instructions:
                si = getattr(inst, "sync_info", None)
                if si is not None and si.on_wait and len(si.on_wait) > cap:
                    waits = list(si.on_wait)
                    extra, keep = waits[:-cap], waits[-cap:]
                    while extra:
                        chunk, extra = extra[:cap], extra[cap:]
                        new_insts.append(
                            mybir.InstNoOp(
                                name=nc.get_next_instruction_name(),
                                sync_info=mybir.SyncInfo(on_wait=chunk, on_update=[]),
                                bass_nofuse=True,
                                engine=inst.engine,
                            )
                        )
                    inst.sync_info = mybir.SyncInfo(
                        on_wait=keep, on_update=list(si.on_update or [])
                    )
                new_insts.append(inst)
            blk.instructions[:] = new_insts


def _dedupe_ldweights(nc):
    """Remove an InstLdweights whose weights AP matches the previous
    InstLdweights on PE, with only InstMatmult in between — the PE array
    still holds those weights, so the reload is redundant. Only drops
    instructions with no semaphore waits/updates."""
    n = 0
    for f in nc.m.functions:
        for blk in f.blocks:
            prev_key = None
            keep = []
            for inst in blk.instructions:
                if getattr(inst, "engine", None) != mybir.EngineType.PE:
                    keep.append(inst)
                    continue
                tn = type(inst).__name__
                if tn == "InstLdweights":
                    w = inst.ins[0]
                    key = (w.memref, w.offset, str(w.ap), str(w.dtype),
                           str(getattr(inst, "perf_mode", None)))
                    si = getattr(inst, "sync_info", None)
                    clean = si is None or (not si.on_wait and not si.on_update)
                    if key == prev_key and clean:
                        n += 1
                        continue  # drop it
                    prev_key = key
                elif tn != "InstMatmult":
                    prev_key = None
                keep.append(inst)
            blk.instructions[:] = keep
    return n


def _build_nc(repeat=None):
    """repeat>1 wraps the whole body in a hardware loop — benchmarking only
    (outputs are simply rewritten each iteration)."""
    if repeat is None:
        repeat = int(os.environ.get("LSTM_BENCH_REPEAT", "1"))
    scheme = list(SCHEME)
    assert len(scheme) == 4 and all(s in "AE" for s in scheme), scheme
    qa_idx = {q: i for i, q in enumerate([q for q in range(4) if scheme[q] == "A"])}
    qe_idx = {q: i for i, q in enumerate([q for q in range(4) if scheme[q] == "E"])}
    nA, nE = len(qa_idx), len(qe_idx)

    nc = bass.Bass()
    f32 = mybir.dt.float32
    bf16 = mybir.dt.bfloat16
    f8e4 = mybir.dt.float8e4
    SIG = mybir.ActivationFunctionType.Sigmoid
    TANH = mybir.ActivationFunctionType.Tanh
    DR = mybir.MatmulPerfMode.DoubleRow
    GATE_FUNC = [SIG, SIG, TANH, SIG]

    if nA:
        xT8 = nc.declare_dram_parameter("xT8", [128, MT, KT, 128], f8e4, isOutput=False)
        hT8 = nc.declare_dram_parameter("hT8", [128, MT, KT, 128], f8e4, isOutput=False)
        w8ih = nc.declare_dram_parameter(
            "w8ih", [128, KT, 2, nA, NQ], f8e4, isOutput=False
        )
        w8hh = nc.declare_dram_parameter(
            "w8hh", [128, KT, 2, nA, NQ], f8e4, isOutput=False
        )
    if nE:
        xTb = nc.declare_dram_parameter("xTb", [128, MT, KT, 128], bf16, isOutput=False)
        hTb = nc.declare_dram_parameter("hTb", [128, MT, KT, 128], bf16, isOutput=False)
        wbih = nc.declare_dram_parameter(
            "wbih", [128, KT, 2, nE, NQ], bf16, isOutput=False
        )
        wbhh = nc.declare_dram_parameter(
            "wbhh", [128, KT, 2, nE, NQ], bf16, isOutput=False
        )
    c0 = nc.declare_dram_parameter("c0", [BL, H], bf16, isOutput=False)
    bjqs = nc.declare_dram_parameter("bjqs", [1, 2, 4, NQ], f32, isOutput=False)
    h1 = nc.declare_dram_parameter("h1", [BL, H], bf16, isOutput=True)
    c1 = nc.declare_dram_parameter("c1", [BL, H], bf16, isOutput=True)

    with tile.TileContext(nc) as tc:
        with (
            tc.tile_pool(name="w", bufs=1) as wpool,
            tc.tile_pool(name="xh", bufs=4) as xhpool,
            tc.tile_pool(name="cc", bufs=4) as cpool,
            tc.tile_pool(name="act", bufs=2) as apool,
            tc.tile_pool(name="outp", bufs=4) as opool,
            tc.tile_pool(name="ps", bufs=8, space="PSUM") as pspool,
        ):
            if nA:
                w8ih_sb = wpool.tile([128, KT, 2, nA, NQ], f8e4)
                w8hh_sb = wpool.tile([128, KT, 2, nA, NQ], f8e4)
            if nE:
                wbih_sb = wpool.tile([128, KT, 2, nE, NQ], bf16)
                wbhh_sb = wpool.tile([128, KT, 2, nE, NQ], bf16)
            bias_sb = wpool.tile([128, 2, 4, NQ], f32)

            if repeat > 1:
                loop_cm = tc.For_i(0, repeat, 1)
                loop_cm.__enter__()

            # Weights on the SP HWDGE queue in exact consumption order
            # (per j: fp8 gates, then bf16 gates; x-matrix then h-matrix).
            # The first fp8 chunk is split so the opening matmul's k-tile
            # pair lands in SBUF as early as possible.
            for j in range(2):
                for qs in range(nA):
                    if j == 0 and qs == 0:
                        nc.sync.dma_start(
                            out=w8ih_sb[:, 0:2, j, qs], in_=w8ih[:, 0:2, j, qs]
                        )
                        nc.sync.dma_start(
                            out=w8ih_sb[:, 2:, j, qs], in_=w8ih[:, 2:, j, qs]
                        )
                    else:
                        nc.sync.dma_start(
                            out=w8ih_sb[:, :, j, qs], in_=w8ih[:, :, j, qs]
                        )
                    nc.sync.dma_start(
                        out=w8hh_sb[:, :, j, qs], in_=w8hh[:, :, j, qs]
                    )
                for qs in range(nE):
                    for kh in range(2):
                        ks = slice(kh * 4, (kh + 1) * 4)
                        nc.sync.dma_start(
                            out=wbih_sb[:, ks, j, qs], in_=wbih[:, ks, j, qs]
                        )
                        nc.sync.dma_start(
                            out=wbhh_sb[:, ks, j, qs], in_=wbhh[:, ks, j, qs]
                        )

            for m in range(MT):
                if nA:
                    xm8 = xhpool.tile([128, KT, 128], f8e4, tag="xm8")
                    hm8 = xhpool.tile([128, KT, 128], f8e4, tag="hm8")
                    nc.scalar.dma_start(out=xm8, in_=xT8[:, m])
                    nc.scalar.dma_start(out=hm8, in_=hT8[:, m])
                if nE:
                    xmb = xhpool.tile([128, KT, 128], bf16, tag="xmb")
                    hmb = xhpool.tile([128, KT, 128], bf16, tag="hmb")
                    nc.scalar.dma_start(out=xmb, in_=xTb[:, m])
                    nc.scalar.dma_start(out=hmb, in_=hTb[:, m])
                if m == 0:
                    # bias isn't needed until the first matmul group finishes;
                    # keep it behind the first x/h tiles on the ACT queue.
                    bj_ap = bjqs[:]
                    bias_bcast = bass.AP(
                        tensor=bj_ap.tensor,
                        offset=bj_ap.offset,
                        ap=[[0, 128]] + list(bj_ap.ap[1:]),
                    )
                    nc.scalar.dma_start(out=bias_sb, in_=bias_bcast)
                for j in range(2):
                    cs = slice(j * NQ, (j + 1) * NQ)
                    rs = slice(m * 128, (m + 1) * 128)

                    c0t = cpool.tile([128, NQ], bf16, tag="c0")
                    nc.scalar.dma_start(out=c0t, in_=c0[rs, cs])

                    ps = [
                        pspool.tile([128, NQ], f32, tag="ps", name=f"ps{q}")
                        for q in range(4)
                    ]
                    # fp8 DoubleRow chains: stationary x/h k-tile pair shared
                    # across the fp8 banks (amortized ldweights when deduped).
                    for kp in range(KT // 2):
                        kslc = slice(2 * kp, 2 * kp + 2)
                        for q in range(4):
                            if scheme[q] != "A":
                                continue
                            nc.tensor.matmul(
                                ps[q],
                                lhsT=xm8[:, kslc],
                                rhs=w8ih_sb[:, kslc, j, qa_idx[q]],
                                start=(kp == 0), stop=False,
                                perf_mode=DR, skip_group_check=True,
                            )
                    for kp in range(KT // 2):
                        kslc = slice(2 * kp, 2 * kp + 2)
                        last = kp == KT // 2 - 1
                        for q in range(4):
                            if scheme[q] != "A":
                                continue
                            nc.tensor.matmul(
                                ps[q],
                                lhsT=hm8[:, kslc],
                                rhs=w8hh_sb[:, kslc, j, qa_idx[q]],
                                start=False, stop=last,
                                perf_mode=DR, skip_group_check=True,
                            )
                    # bf16 chains
                    for kt in range(KT):
                        for q in range(4):
                            if scheme[q] != "E":
                                continue
                            nc.tensor.matmul(
                                ps[q],
                                lhsT=xmb[:, kt],
                                rhs=wbih_sb[:, kt, j, qe_idx[q]],
                                start=(kt == 0), stop=False,
                                skip_group_check=True,
                            )
                    for kt in range(KT):
                        last = kt == KT - 1
                        for q in range(4):
                            if scheme[q] != "E":
                                continue
                            nc.tensor.matmul(
                                ps[q],
                                lhsT=hmb[:, kt],
                                rhs=wbhh_sb[:, kt, j, qe_idx[q]],
                                start=False, stop=last,
                                skip_group_check=True,
                            )

                    g = [
                        apool.tile([128, NQ], f32, tag=f"g{q}", name=f"g{q}")
                        for q in range(4)
                    ]
                    for q in range(4):
                        # bias add on DVE (PSUM -> SBUF); ACT applies the
                        # nonlinearity, folding the fp8 weight prescale back
                        # out via its input scale operand.
                        nc.vector.tensor_add(
                            out=g[q], in0=ps[q], in1=bias_sb[:, j, q]
                        )
                        nc.scalar.activation(
                            out=g[q], in_=g[q], func=GATE_FUNC[q],
                            scale=(1.0 / WSCALE) if scheme[q] == "A" else 1.0,
                        )

                    gi, gf, gg, go = g
                    nc.vector.tensor_mul(out=gi, in0=gi, in1=gg)   # sig(i)*tanh(g)
                    nc.vector.tensor_mul(out=gf, in0=gf, in1=c0t)  # sig(f)*c0
                    c1t = opool.tile([128, NQ], bf16, tag="c1")
                    nc.vector.tensor_add(out=c1t, in0=gi, in1=gf)
                    tc1 = apool.tile([128, NQ], f32, tag="tc1")
                    nc.scalar.activation(out=tc1, in_=c1t, func=TANH)
                    h1t = opool.tile([128, NQ], bf16, tag="h1")
                    nc.vector.tensor_mul(out=h1t, in0=go, in1=tc1)

                    nc.sync.dma_start(out=c1[rs, cs], in_=c1t)
                    nc.sync.dma_start(out=h1[rs, cs], in_=h1t)

            if repeat > 1:
                loop_cm.__exit__(None, None, None)

    _split_excess_waits(nc)
    if os.environ.get("LSTM_LDW_DEDUPE", "0") == "1":
        _dedupe_ldweights(nc)
    return nc


_NC = None


def _get_nc():
    global _NC
    if _NC is None:
        _NC = _build_nc()
    return _NC


def _prep_xT4(x, dtype):
    """[B, 1024] fp32 -> [N_CORES][128, MT, KT, 128] per-core arrays."""
    v = x.reshape(N_CORES, MT, 128, KT, 128)  # [c, m, b, kt, p]
    v = v.transpose(0, 4, 1, 3, 2)            # [c, p, m, kt, b]
    v = v.astype(dtype)
    return [np.ascontiguousarray(v[c]) for c in range(N_CORES)]


def _prep_w5(w, gates, dtype, scale=1.0):
    """[4096, 1024] fp32 -> [128, KT, 2, len(gates), 512] (replicated)."""
    v = w.reshape(4, 2, NQ, KT, 128)[gates]  # [q, j, s, kt, p]
    if scale != 1.0:
        v = v * scale
    v = v.transpose(4, 3, 1, 0, 2)           # [p, kt, j, q, s]
    return np.ascontiguousarray(v.astype(dtype))


def _make_in_maps(input, h_0, c_0, W_ih, b_ih, W_hh, b_hh):
    scheme = list(SCHEME)
    gates_a = [q for q in range(4) if scheme[q] == "A"]
    gates_e = [q for q in range(4) if scheme[q] == "E"]

    x = np.asarray(input, dtype=np.float32)
    h0 = np.asarray(h_0, dtype=np.float32)
    c0 = np.asarray(c_0, dtype=np.float32)
    wih = np.asarray(W_ih, dtype=np.float32)
    whh = np.asarray(W_hh, dtype=np.float32)
    b = (np.asarray(b_ih, dtype=np.float32) + np.asarray(b_hh, dtype=np.float32))
    bq = b.reshape(4, 2, NQ).copy()
    for q in gates_a:
        bq[q] *= WSCALE
    bjqs = np.ascontiguousarray(bq.transpose(1, 0, 2)[None])  # [1, 2(j), 4(q), 512]

    base = {
        "bjqs": bjqs,
    }
    if gates_a:
        base["w8ih"] = _prep_w5(wih, gates_a, F8E4, WSCALE)
        base["w8hh"] = _prep_w5(whh, gates_a, F8E4, WSCALE)
        xs8 = _prep_xT4(x, F8E4)
        hs8 = _prep_xT4(h0, F8E4)
    if gates_e:
        base["wbih"] = _prep_w5(wih, gates_e, BF16)
        base["wbhh"] = _prep_w5(whh, gates_e, BF16)
        xsb = _prep_xT4(x, BF16)
        hsb = _prep_xT4(h0, BF16)
    c0s = c0.reshape(N_CORES, BL, H).astype(BF16)

    in_maps = []
    for c in range(N_CORES):
        m = dict(base)
        m["c0"] = np.ascontiguousarray(c0s[c])
        if gates_a:
            m["xT8"] = xs8[c]
            m["hT8"] = hs8[c]
        if gates_e:
            m["xTb"] = xsb[c]
            m["hTb"] = hsb[c]
        in_maps.append(m)
    return in_maps


def kernel(input, h_0, c_0, W_ih, b_ih, W_hh, b_hh):
    in_maps = _make_in_maps(input, h_0, c_0, W_ih, b_ih, W_hh, b_hh)
    nc = _get_nc()
    res = run_bass_kernel_spmd(nc, in_maps, core_ids=list(range(N_CORES)))
    h_1 = np.concatenate(
        [res.results[c]["h1"].astype(np.float32) for c in range(N_CORES)], axis=0
    )
    c_1 = np.concatenate(
        [res.results[c]["c1"].astype(np.float32) for c in range(N_CORES)], axis=0
    )
    return (h_1, c_1)
